# revision 59
# baseline (speedup 1.0000x reference)
"""Capsule dynamic-routing kernel for 8 TRN2 NeuronCores.

Problem: nn_CapsuleRouting — p:(16,32,16,14,14), W_ij:(32,4,4,32), 3 routing
iterations, returns (v:(16,32,16,14,14), a:(16,32,14,14)).

Sharding: data-parallel over batch (2 batch elems per core, 8 cores).

Per-core design:
  - 392 positions (2 batch x 196 hw) in 4 chunks of 98 partitions.
  - SBUF layout: partitions = positions; u votes stored [98, B=32, ik=16, C=32]
    bf16.  Softmax coeffs c[B,C,pos] and squashed votes v[C,ik,pos] broadcast
    along free dims with step-0 APs (no replication needed).
  - Vote einsum on PE with a host-built block-diagonal W (K=128, all B per
    matmul); a plain stacked W contracts (B,j) to give iter0's s0 for free.
  - Reductions over B (weighted sum -> s) and over ik (r update) via PSUM
    accumulation with identity-lhsT matmuls on PE, pipelined behind the DVE
    bf16 2x multiply passes (w2 = c*u or u*v).
  - Small math (softmax exp, squash, reciprocal) on ACT/DVE per 2-chunk
    group so groups pipeline against each other's big passes.
"""

import os
import numpy as np
from contextlib import ExitStack
EXP = os.environ.get("KEXP", "")

P = 4
PP = 16
B = 32
C = 32
H = 14
WW = 14
POS = H * WW          # 196
HALF = POS // 2       # 98
BLOC = 2              # batch elems per core
NCH = 4               # chunks per core: (batch, half)
GRP = 1               # chunks per small-op group
ITERS = 3
EPS = 1e-5
NCORES = 8


def _build(debug=False, reps=1, serialize=False):
    import concourse.bass as bass
    import concourse.bacc as bacc
    import concourse.tile as tile
    from concourse import mybir

    f32 = mybir.dt.float32
    bf16 = mybir.dt.bfloat16
    AX = mybir.AxisListType
    AF = mybir.ActivationFunctionType

    nc = bacc.Bacc()
    p_ext = nc.declare_dram_parameter("p", [BLOC, B, PP, H, WW], bf16, isOutput=False)
    wbd_ext = nc.declare_dram_parameter("Wbd", [128, B * 128], bf16, isOutput=False)
    wsum_ext = nc.declare_dram_parameter("Wsum", [128, 128], bf16, isOutput=False)
    id_ext = nc.declare_dram_parameter("ident", [128, 128], f32, isOutput=False)
    v_ext = nc.declare_dram_parameter("v_out", [BLOC, C, PP, POS], f32, isOutput=True)
    a_ext = nc.declare_dram_parameter("a_out", [BLOC, C, POS], f32, isOutput=True)
    if debug:
        du_ext = nc.declare_dram_parameter(
            "dbg_u", [NCH, 98, B, PP, C], bf16, isOutput=True)
        ds_ext = nc.declare_dram_parameter(
            "dbg_s", [98, NCH, PP, C], f32, isOutput=True)

    def bcast(sl, axis, count):
        """Insert a step-0 (broadcast) dim into AP `sl` at free-dim position
        `axis` (0 = first free dim)."""
        ap = list(sl.ap)
        ap.insert(1 + axis, [0, count])
        return bass.AP(tensor=sl.tensor, offset=sl.offset, ap=ap)

    with tile.TileContext(nc) as tc, ExitStack() as ctx:
        singles = ctx.enter_context(tc.tile_pool(name="singles", bufs=1))
        upool = ctx.enter_context(tc.tile_pool(name="upool", bufs=4))
        w2pool = ctx.enter_context(tc.tile_pool(name="w2pool", bufs=3))
        small = ctx.enter_context(tc.tile_pool(name="small", bufs=1))
        outp = ctx.enter_context(tc.tile_pool(name="outp", bufs=2))
        mmps = ctx.enter_context(tc.tile_pool(name="mmps", bufs=2, space="PSUM"))
        sps = ctx.enter_context(tc.tile_pool(name="sps", bufs=3, space="PSUM"))
        tps = ctx.enter_context(tc.tile_pool(name="tps", bufs=1, space="PSUM"))

        # --- constants ---
        id_f = singles.tile([128, 128], f32)
        nc.sync.dma_start(out=id_f, in_=id_ext[:, :])
        id_b = singles.tile([128, 128], bf16)
        nc.scalar.copy(id_b, id_f)
        idb98 = id_b[:98, :98]
        idf98 = id_f[:98, :98]
        eps_t = singles.tile([98, 1], f32)
        nc.vector.memset(eps_t, EPS)
        wsum = singles.tile([128, 128], bf16)
        nc.sync.dma_start(out=wsum, in_=wsum_ext[:, :])

        # --- persistent state (chunk index is a free dim; ops slice groups) ---
        u = [upool.tile([98, B, PP, C], bf16, tag="u", name=f"u{i}")
             for i in range(NCH)]
        r_all = small.tile([98, NCH, B, C], bf16)      # routing logits
        c_all = small.tile([98, NCH, B, C], bf16)      # softmax coeffs
        s_all = small.tile([98, NCH, PP, C], f32)      # s, then v (in-place)
        vbf_all = small.tile([98, NCH, PP, C], bf16)   # v in bf16
        n2_all = small.tile([98, NCH, C], f32)
        nrm_all = small.tile([98, NCH, C], f32)
        sc_all = small.tile([98, NCH, C], f32)
        d_all = small.tile([98, NCH, B], f32)

        def squash(it, c0, cn, sp_tiles=None):
            """s -> v (into s_all) for chunks [c0, c0+cn); sets n2/nrm/sc.
            With sp_tiles, reads s from the per-chunk accum PSUM directly
            (skips the ACT psum->SBUF copy on the critical chain)."""
            ssl = s_all[:, c0:c0 + cn, :, :]
            # scratch for squares: alias the dead B<16 half of c_all (current
            # iter's c is already consumed; next softmax rewrites it fully)
            sq = c_all[:, c0:c0 + cn, :PP, :]
            scale = (1.0 / B) if it == 0 else 1.0
            n2 = n2_all[:, c0:c0 + cn, :]
            for k, sp in enumerate(sp_tiles):
                nc.scalar.activation(sq[:, k, :, :], sp, AF.Square,
                                     scale=scale)
                nc.vector.tensor_reduce(
                    n2_all[:, c0 + k:c0 + k + 1, :],
                    sq[:, k:k + 1, :, :].rearrange("q n i c -> q n c i"),
                    axis=AX.X, op=mybir.AluOpType.add,
                )
            nrm = nrm_all[:, c0:c0 + cn, :]
            sc = sc_all[:, c0:c0 + cn, :]
            nc.scalar.activation(nrm, n2, AF.Sqrt, bias=eps_t)
            nc.gpsimd.tensor_scalar_add(sc, n2, 1.0)
            nc.gpsimd.tensor_mul(sc, sc, nrm)          # (1+n2)*nrm
            nc.vector.reciprocal(sc, sc)
            nc.gpsimd.tensor_mul(sc, sc, n2)           # n2/((1+n2)nrm)
            if it == 0:
                # v = (s0_psum/32) * sc: fold 1/32 into a scaled sc copy
                sc32 = d_all[:, c0:c0 + cn, :]
                nc.scalar.activation(sc32, sc, AF.Copy, scale=1.0 / B)
                for k, sp in enumerate(sp_tiles):
                    nc.vector.tensor_mul(
                        s_all[:, c0 + k, :, :], sp,
                        bcast(d_all[:, c0 + k, :], 0, PP))
            else:
                for k, sp in enumerate(sp_tiles):
                    nc.vector.tensor_mul(
                        s_all[:, c0 + k, :, :], sp,
                        bcast(sc_all[:, c0 + k, :], 0, PP))
            if it != ITERS - 1:
                nc.scalar.copy(vbf_all[:, c0:c0 + cn, :, :], ssl)

        def softmax(c0, cn):
            rsl = r_all[:, c0:c0 + cn, :, :]
            csl = c_all[:, c0:c0 + cn, :, :]
            d = d_all[:, c0:c0 + cn, :]
            nc.scalar.activation(csl, rsl, AF.Exp)
            nc.vector.tensor_reduce(d, csl, axis=AX.X, op=mybir.AluOpType.add)
            nc.vector.reciprocal(d, d)
            nc.gpsimd.tensor_mul(csl, csl, bcast(d, 2, C))

        def emit_outputs(c0, cn):
            asl = nrm_all[:, c0:c0 + cn, :]
            # a = sqrt((n2/(1+n2))^2 + eps);  n2/(1+n2) = sc*nrm
            nc.vector.tensor_mul(asl, sc_all[:, c0:c0 + cn, :], asl)
            nc.vector.tensor_mul(asl, asl, asl)
            nc.scalar.activation(asl, asl, AF.Sqrt, bias=eps_t)
            for ch in range(c0, c0 + cn):
                bi, hf = divmod(ch, 2)
                vdst = v_ext[bi].rearrange("c (i k) s -> k c i s", i=P, k=P)
                for q in range(P):
                    tp = tps.tile([128, 98], f32, tag="tp")
                    nc.tensor.transpose(
                        tp,
                        s_all[:, ch, q * P:(q + 1) * P, :]
                        .rearrange("q a b -> q (a b)"),
                        idf98,
                    )
                    vt = outp.tile([128, 98], f32, tag="vt")
                    nc.scalar.copy(vt, tp)
                    nc.sync.dma_start(
                        out=vdst[:, :, q, hf * HALF:(hf + 1) * HALF],
                        in_=vt,
                    )
                tp = tps.tile([128, 98], f32, tag="tp")
                nc.tensor.transpose(tp[:C, :], nrm_all[:, ch, :], idf98)
                at = outp.tile([128, 98], f32, tag="vt")
                nc.scalar.copy(at[:C, :], tp[:C, :])
                nc.sync.dma_start(
                    out=a_ext[bi, :, hf * HALF:(hf + 1) * HALF],
                    in_=at[:C, :],
                )

        for _rep in range(reps):
            # --- phase 1: vote einsum ---
            if True:
                wbd = w2pool.tile([128, B * 128], bf16, tag="w2",
                                  name=f"wbd{_rep}")
                nc.sync.dma_start(out=wbd, in_=wbd_ext[:, :])
                s0_tiles = []
                if serialize and _rep > 0:
                    # hard cross-rep serializer: wbd corner <- 0 * prev output
                    # (unused id_f corner as bounce; outside the idf98 slice)
                    nc.sync.dma_start(out=id_f[0:1, 120:128],
                                      in_=a_ext[0, 0, 0:8])
                    nc.vector.tensor_scalar_mul(
                        wbd[0:1, 512:520],
                        id_f[0:1, 120:124].bitcast(bf16), 0.0)
                for ch in range(NCH):
                    bi, hf = divmod(ch, 2)
                    src = p_ext[bi].rearrange(
                        "b (i j) h w -> b j i (h w)", i=P, j=P)
                    p_b = w2pool.tile([128, P, HALF], bf16, tag="w2",
                                      name=f"pb{_rep}_{ch}")
                    for i in range(P):
                        nc.sync.dma_start(
                            out=p_b[:, i, :],
                            in_=src[:, :, i, hf * HALF:(hf + 1) * HALF],
                        )
                    s0 = sps.tile([98, P, 128], f32, tag="sp",
                                  name=f"s0_{ch}")
                    for i in range(P):
                        nc.tensor.matmul(s0[:, i, :], p_b[:, i, :], wsum,
                                         start=True, stop=True)
                    s0_tiles.append(s0.rearrange("q i n -> q (i n)"))
                    for i in range(P):
                        for hp in range(4):
                            ps = mmps.tile([98, 8, P, C], f32, tag="mm")
                            for t in range(2):
                                pc = hp * 2 + t
                                nc.tensor.matmul(
                                    ps[:, t * 4:(t + 1) * 4, :, :],
                                    p_b[:, i, :],
                                    wbd[:, pc * 512:(pc + 1) * 512],
                                    start=True, stop=True,
                                )
                            dst = u[ch][:, hp * 8:(hp + 1) * 8,
                                        i * P:(i + 1) * P, :]
                            if hp % 2 == 0:
                                nc.vector.tensor_copy(dst, ps)
                            else:
                                nc.scalar.copy(dst, ps)

            # --- routing iterations (iter0 s came from the einsum) ---
            for it in range(ITERS):
                grp = 1 if it == ITERS - 1 else GRP
                for g2 in range(NCH // grp):
                    chs = range(g2 * grp, (g2 + 1) * grp)
                    if it == 0:
                        sp_tiles = [s0_tiles[ch] for ch in chs]
                    else:
                        sp_tiles = []
                        for ch in chs:
                            sp = sps.tile([98, PP, C], f32, tag="sp",
                                          name=f"sp{it}_{ch}")
                            sp_tiles.append(sp)
                            for sb in range(2):       # 16 B-caps per sub
                                w2 = w2pool.tile([98, 16, PP, C], bf16,
                                                 tag="w2", name=f"w2s{ch}{sb}")
                                csl = c_all[:, ch, sb * 16:(sb + 1) * 16, :]
                                if "nomul" in EXP:
                                    nc.vector.tensor_mul(
                                        w2[:, :, 0, :],
                                        u[ch][:, sb * 16:(sb + 1) * 16, 0, :],
                                        csl)
                                else:
                                    nc.vector.tensor_mul(
                                        w2, u[ch][:, sb * 16:(sb + 1) * 16, :, :],
                                        bcast(csl, 1, PP),
                                    )
                                for b16 in range(16):
                                    if "noacc" in EXP and not (
                                            sb == 0 and b16 == 0):
                                        continue
                                    nc.tensor.matmul(
                                        sp[:, :, :], idb98, w2[:, b16, :, :],
                                        start=(sb == 0 and b16 == 0),
                                        stop=True if "noacc" in EXP else
                                        (sb == 1 and b16 == 15),
                                    )
                    squash(it, g2 * grp, grp, sp_tiles)
                    if it == ITERS - 1:
                        emit_outputs(g2 * grp, grp)
                        continue
                    for ch in chs:
                        dp = mmps.tile([98, B, C], f32, tag="mm",
                                       name=f"dp{it}_{ch}")
                        for sb in range(2):
                            w2 = w2pool.tile([98, 16, PP, C], bf16,
                                             tag="w2", name=f"w2d{ch}{sb}")
                            vsl = vbf_all[:, ch, :, :]
                            if "nomul" in EXP:
                                nc.vector.tensor_mul(
                                    w2[:, 0, :, :],
                                    u[ch][:, sb * 16, :, :], vsl)
                            else:
                                nc.vector.tensor_mul(
                                    w2, u[ch][:, sb * 16:(sb + 1) * 16, :, :],
                                    bcast(vsl, 0, 16),
                                )
                            for ik in range(PP):
                                if "noacc" in EXP and ik > 0:
                                    continue
                                nc.tensor.matmul(
                                    dp[:, sb * 16:(sb + 1) * 16, :],
                                    idb98, w2[:, :, ik, :],
                                    start=(ik == 0),
                                    stop=True if "noacc" in EXP else
                                    (ik == PP - 1),
                                )
                        if it == 0:
                            nc.scalar.copy(r_all[:, ch, :, :], dp)
                        else:
                            nc.vector.tensor_add(
                                r_all[:, ch, :, :], r_all[:, ch, :, :], dp)
                    softmax(g2 * grp, grp)


            if debug:
                for ch in range(NCH):
                    nc.sync.dma_start(out=du_ext[ch], in_=u[ch])
                nc.sync.dma_start(out=ds_ext[:, :, :, :], in_=s_all)

    nc.compile()
    return nc


_NC_CACHE = None


def kernel(p, a, W_ij):
    global _NC_CACHE
    from concourse.bass_utils import run_bass_kernel_spmd

    if _NC_CACHE is None:
        _NC_CACHE = _build()
    nc = _NC_CACHE

    import ml_dtypes
    p = np.asarray(p, dtype=np.float32).astype(ml_dtypes.bfloat16)
    Wf = np.asarray(W_ij, dtype=np.float32)          # (B, j, k, C)
    wsum = Wf.reshape(128, 128).astype(ml_dtypes.bfloat16)
    wbd = np.zeros((128, B * 128), dtype=np.float32)
    for b in range(B):
        wbd[b * P:(b + 1) * P, b * 128:(b + 1) * 128] = Wf[b].reshape(P, 128)
    wbd = wbd.astype(ml_dtypes.bfloat16)
    ident = np.eye(128, dtype=np.float32)
    in_maps = [
        {"p": p[2 * i:2 * i + 2], "Wbd": wbd, "Wsum": wsum, "ident": ident}
        for i in range(NCORES)
    ]
    res = run_bass_kernel_spmd(nc, in_maps, core_ids=list(range(NCORES)))
    v_parts = [res.results[i]["v_out"].reshape(BLOC, C, PP, H, WW)
               for i in range(NCORES)]
    a_parts = [res.results[i]["a_out"].reshape(BLOC, C, H, WW)
               for i in range(NCORES)]
    v_full = np.concatenate(v_parts, axis=0)
    a_full = np.concatenate(a_parts, axis=0)
    return v_full, a_full


# revision 65
# speedup vs baseline: 1.1745x; 1.1745x over previous
"""Capsule dynamic-routing kernel for 8 TRN2 NeuronCores.

Problem: nn_CapsuleRouting — p:(16,32,16,14,14), W_ij:(32,4,4,32), 3 routing
iterations, returns (v:(16,32,16,14,14), a:(16,32,14,14)).

Sharding: data-parallel over batch (2 batch elems per core, 8 cores).

Per-core design:
  - 392 positions (2 batch x 196 hw) in 4 chunks of 98 partitions.
  - SBUF layout: partitions = positions; u votes stored [98, B=32, ik=16, C=32]
    bf16.  Softmax coeffs c[B,C,pos] and squashed votes v[C,ik,pos] broadcast
    along free dims with step-0 APs (no replication needed).
  - Vote einsum on PE with a host-built block-diagonal W (K=128, all B per
    matmul); a plain stacked W contracts (B,j) to give iter0's s0 for free.
  - Reductions over B (weighted sum -> s) and over ik (r update) via PSUM
    accumulation with identity-lhsT matmuls on PE, pipelined behind the DVE
    bf16 2x multiply passes (w2 = c*u or u*v).
  - Small math (softmax exp, squash, reciprocal) on ACT/DVE per 2-chunk
    group so groups pipeline against each other's big passes.
"""

import os
import numpy as np
from contextlib import ExitStack
EXP = os.environ.get("KEXP", "")

P = 4
PP = 16
B = 32
C = 32
H = 14
WW = 14
POS = H * WW          # 196
HALF = POS // 2       # 98
BLOC = 2              # batch elems per core
NCH = 4               # chunks per core: (batch, half)
GRP = 1               # chunks per small-op group
ITERS = 3
EPS = 1e-5
NCORES = 8


def _build(debug=False, reps=1, serialize=False):
    import concourse.bass as bass
    import concourse.bacc as bacc
    import concourse.tile as tile
    from concourse import mybir

    f32 = mybir.dt.float32
    bf16 = mybir.dt.bfloat16
    AX = mybir.AxisListType
    AF = mybir.ActivationFunctionType

    nc = bacc.Bacc()
    p_ext = nc.declare_dram_parameter("p", [BLOC, B, PP, H, WW], bf16, isOutput=False)
    wbd_ext = nc.declare_dram_parameter("Wbd", [128, B * 128], bf16, isOutput=False)
    wsum_ext = nc.declare_dram_parameter("Wsum", [128, 128], bf16, isOutput=False)
    id_ext = nc.declare_dram_parameter("ident", [128, 128], f32, isOutput=False)
    v_ext = nc.declare_dram_parameter("v_out", [BLOC, C, PP, POS], f32, isOutput=True)
    a_ext = nc.declare_dram_parameter("a_out", [BLOC, C, POS], f32, isOutput=True)
    if debug:
        du_ext = nc.declare_dram_parameter(
            "dbg_u", [NCH, 98, B, PP, C], bf16, isOutput=True)
        ds_ext = nc.declare_dram_parameter(
            "dbg_s", [98, NCH, PP, C], f32, isOutput=True)

    def bcast(sl, axis, count):
        """Insert a step-0 (broadcast) dim into AP `sl` at free-dim position
        `axis` (0 = first free dim)."""
        ap = list(sl.ap)
        ap.insert(1 + axis, [0, count])
        return bass.AP(tensor=sl.tensor, offset=sl.offset, ap=ap)

    with tile.TileContext(nc) as tc, ExitStack() as ctx:
        singles = ctx.enter_context(tc.tile_pool(name="singles", bufs=1))
        upool = ctx.enter_context(tc.tile_pool(name="upool", bufs=4))
        w2pool = ctx.enter_context(tc.tile_pool(name="w2pool", bufs=3))
        small = ctx.enter_context(tc.tile_pool(name="small", bufs=1))
        outp = ctx.enter_context(tc.tile_pool(name="outp", bufs=2))
        mmps = ctx.enter_context(tc.tile_pool(name="mmps", bufs=2, space="PSUM"))
        sps = ctx.enter_context(tc.tile_pool(name="sps", bufs=3, space="PSUM"))
        tps = ctx.enter_context(tc.tile_pool(name="tps", bufs=1, space="PSUM"))

        # --- constants ---
        id_f = singles.tile([128, 128], f32)
        nc.sync.dma_start(out=id_f, in_=id_ext[:, :])
        id_b = singles.tile([128, 128], bf16)
        nc.scalar.copy(id_b, id_f)
        idb98 = id_b[:98, :98]
        idf98 = id_f[:98, :98]
        eps_t = singles.tile([98, 1], f32)
        nc.vector.memset(eps_t, EPS)
        wsum = singles.tile([128, 128], bf16)
        nc.sync.dma_start(out=wsum, in_=wsum_ext[:, :])

        # --- persistent state (chunk index is a free dim; ops slice groups) ---
        u = [upool.tile([98, B, PP, C], bf16, tag="u", name=f"u{i}")
             for i in range(NCH)]
        r_all = small.tile([98, NCH, B, C], bf16)      # routing logits
        c_all = small.tile([98, NCH, B, C], bf16)      # softmax coeffs
        s_all = small.tile([98, NCH, PP, C], f32)      # s, then v (in-place)
        vbf_all = small.tile([98, NCH, PP, C], bf16)   # v in bf16
        n2_all = small.tile([98, NCH, C], f32)
        nrm_all = small.tile([98, NCH, C], f32)
        sc_all = small.tile([98, NCH, C], f32)
        d_all = small.tile([98, NCH, B], f32)

        def squash(it, c0, cn, sp_tiles=None):
            """s -> v (into s_all) for chunks [c0, c0+cn); sets n2/nrm/sc.
            With sp_tiles, reads s from the per-chunk accum PSUM directly
            (skips the ACT psum->SBUF copy on the critical chain)."""
            ssl = s_all[:, c0:c0 + cn, :, :]
            # scratch for squares: alias the dead B<16 half of c_all (current
            # iter's c is already consumed; next softmax rewrites it fully)
            sq = c_all[:, c0:c0 + cn, :PP, :]
            scale = (1.0 / B) if it == 0 else 1.0
            n2 = n2_all[:, c0:c0 + cn, :]
            for k, sp in enumerate(sp_tiles):
                nc.scalar.activation(sq[:, k, :, :], sp, AF.Square,
                                     scale=scale)
                nc.vector.tensor_reduce(
                    n2_all[:, c0 + k:c0 + k + 1, :],
                    sq[:, k:k + 1, :, :].rearrange("q n i c -> q n c i"),
                    axis=AX.X, op=mybir.AluOpType.add,
                )
            nrm = nrm_all[:, c0:c0 + cn, :]
            sc = sc_all[:, c0:c0 + cn, :]
            nc.scalar.activation(nrm, n2, AF.Sqrt, bias=eps_t)
            nc.gpsimd.tensor_scalar_add(sc, n2, 1.0)
            nc.gpsimd.tensor_mul(sc, sc, nrm)          # (1+n2)*nrm
            nc.vector.reciprocal(sc, sc)
            nc.gpsimd.tensor_mul(sc, sc, n2)           # n2/((1+n2)nrm)
            if it == 0:
                # v = (s0_psum/32) * sc: fold 1/32 into a scaled sc copy
                sc32 = d_all[:, c0:c0 + cn, :]
                nc.scalar.activation(sc32, sc, AF.Copy, scale=1.0 / B)
                for k, sp in enumerate(sp_tiles):
                    nc.vector.tensor_mul(
                        s_all[:, c0 + k, :, :], sp,
                        bcast(d_all[:, c0 + k, :], 0, PP))
            else:
                for k, sp in enumerate(sp_tiles):
                    nc.vector.tensor_mul(
                        s_all[:, c0 + k, :, :], sp,
                        bcast(sc_all[:, c0 + k, :], 0, PP))
            if it != ITERS - 1:
                nc.scalar.copy(vbf_all[:, c0:c0 + cn, :, :], ssl)

        def softmax(c0, cn):
            rsl = r_all[:, c0:c0 + cn, :, :]
            csl = c_all[:, c0:c0 + cn, :, :]
            d = d_all[:, c0:c0 + cn, :]
            nc.scalar.activation(csl, rsl, AF.Exp)
            nc.vector.tensor_reduce(d, csl, axis=AX.X, op=mybir.AluOpType.add)
            nc.vector.reciprocal(d, d)
            nc.gpsimd.tensor_mul(csl, csl, bcast(d, 2, C))

        def emit_outputs(c0, cn):
            asl = nrm_all[:, c0:c0 + cn, :]
            # a = sqrt((n2/(1+n2))^2 + eps);  n2/(1+n2) = sc*nrm
            nc.vector.tensor_mul(asl, sc_all[:, c0:c0 + cn, :], asl)
            nc.vector.tensor_mul(asl, asl, asl)
            nc.scalar.activation(asl, asl, AF.Sqrt, bias=eps_t)
            for ch in range(c0, c0 + cn):
                bi, hf = divmod(ch, 2)
                vdst = v_ext[bi].rearrange("c (i k) s -> k c i s", i=P, k=P)
                for q in range(P):
                    tp = tps.tile([128, 98], f32, tag="tp")
                    nc.tensor.transpose(
                        tp,
                        s_all[:, ch, q * P:(q + 1) * P, :]
                        .rearrange("q a b -> q (a b)"),
                        idf98,
                    )
                    vt = outp.tile([128, 98], f32, tag="vt")
                    nc.scalar.copy(vt, tp)
                    nc.sync.dma_start(
                        out=vdst[:, :, q, hf * HALF:(hf + 1) * HALF],
                        in_=vt,
                    )
                tp = tps.tile([128, 98], f32, tag="tp")
                nc.tensor.transpose(tp[:C, :], nrm_all[:, ch, :], idf98)
                at = outp.tile([128, 98], f32, tag="vt")
                nc.scalar.copy(at[:C, :], tp[:C, :])
                nc.sync.dma_start(
                    out=a_ext[bi, :, hf * HALF:(hf + 1) * HALF],
                    in_=at[:C, :],
                )

        for _rep in range(reps):
            # --- phase 1: vote einsum ---
            if True:
                wbd = w2pool.tile([128, B * 128], bf16, tag="w2",
                                  name=f"wbd{_rep}")
                nc.sync.dma_start(out=wbd, in_=wbd_ext[:, :])
                s0_tiles = []
                if serialize and _rep > 0:
                    # hard cross-rep serializer: wbd corner <- 0 * prev output
                    # (unused id_f corner as bounce; outside the idf98 slice)
                    nc.sync.dma_start(out=id_f[0:1, 120:128],
                                      in_=a_ext[0, 0, 0:8])
                    nc.vector.tensor_scalar_mul(
                        wbd[0:1, 512:520],
                        id_f[0:1, 120:124].bitcast(bf16), 0.0)
                for ch in range(NCH):
                    bi, hf = divmod(ch, 2)
                    src = p_ext[bi].rearrange(
                        "b (i j) h w -> b j i (h w)", i=P, j=P)
                    p_b = w2pool.tile([128, P, HALF], bf16, tag="w2",
                                      name=f"pb{_rep}_{ch}")
                    for i in range(P):
                        nc.sync.dma_start(
                            out=p_b[:, i, :],
                            in_=src[:, :, i, hf * HALF:(hf + 1) * HALF],
                        )
                    s0 = sps.tile([98, P, 128], f32, tag="sp",
                                  name=f"s0_{ch}")
                    for i in range(P):
                        nc.tensor.matmul(s0[:, i, :], p_b[:, i, :], wsum,
                                         start=True, stop=True)
                    s0_tiles.append(s0.rearrange("q i n -> q (i n)"))
                    for i in range(P):
                        for hp in range(4):
                            ps = mmps.tile([98, 8, P, C], f32, tag="mm")
                            for t in range(2):
                                pc = hp * 2 + t
                                nc.tensor.matmul(
                                    ps[:, t * 4:(t + 1) * 4, :, :],
                                    p_b[:, i, :],
                                    wbd[:, pc * 512:(pc + 1) * 512],
                                    start=True, stop=True,
                                )
                            dst = u[ch][:, hp * 8:(hp + 1) * 8,
                                        i * P:(i + 1) * P, :]
                            if hp % 2 == 0:
                                nc.vector.tensor_copy(dst, ps)
                            else:
                                nc.scalar.copy(dst, ps)

            # --- routing iterations (iter0 s came from the einsum) ---
            for it in range(ITERS):
                grp = 1 if it == ITERS - 1 else GRP
                for g2 in range(NCH // grp):
                    chs = range(g2 * grp, (g2 + 1) * grp)
                    if it == 0:
                        sp_tiles = [s0_tiles[ch] for ch in chs]
                    else:
                        sp_tiles = []
                        SUBS = [(0, 8), (8, 8), (16, 16)]
                        for ch in chs:
                            sp = sps.tile([98, PP, C], f32, tag="sp",
                                          name=f"sp{it}_{ch}")
                            sp_tiles.append(sp)
                            for si, (b0, sz) in enumerate(SUBS):
                                w2 = w2pool.tile([98, sz, PP, C], bf16,
                                                 tag="w2", name=f"w2s{ch}{si}")
                                csl = c_all[:, ch, b0:b0 + sz, :]
                                nc.vector.tensor_mul(
                                    w2, u[ch][:, b0:b0 + sz, :, :],
                                    bcast(csl, 1, PP),
                                )
                                for bb in range(sz):
                                    nc.tensor.matmul(
                                        sp[:, :, :], idb98, w2[:, bb, :, :],
                                        start=(b0 == 0 and bb == 0),
                                        stop=(b0 + bb == B - 1),
                                    )
                    squash(it, g2 * grp, grp, sp_tiles)
                    if it == ITERS - 1:
                        emit_outputs(g2 * grp, grp)
                        continue
                    for ch in chs:
                        dp = mmps.tile([98, B, C], f32, tag="mm",
                                       name=f"dp{it}_{ch}")
                        for sb in range(2):
                            w2 = w2pool.tile([98, 16, PP, C], bf16,
                                             tag="w2", name=f"w2d{ch}{sb}")
                            vsl = vbf_all[:, ch, :, :]
                            if "nomul" in EXP:
                                nc.vector.tensor_mul(
                                    w2[:, 0, :, :],
                                    u[ch][:, sb * 16, :, :], vsl)
                            else:
                                nc.vector.tensor_mul(
                                    w2, u[ch][:, sb * 16:(sb + 1) * 16, :, :],
                                    bcast(vsl, 0, 16),
                                )
                            for ik in range(PP):
                                if "noacc" in EXP and ik > 0:
                                    continue
                                nc.tensor.matmul(
                                    dp[:, sb * 16:(sb + 1) * 16, :],
                                    idb98, w2[:, :, ik, :],
                                    start=(ik == 0),
                                    stop=True if "noacc" in EXP else
                                    (ik == PP - 1),
                                )
                        if it == 0:
                            nc.scalar.copy(r_all[:, ch, :, :], dp)
                        else:
                            nc.vector.tensor_add(
                                r_all[:, ch, :, :], r_all[:, ch, :, :], dp)
                    softmax(g2 * grp, grp)


            if debug:
                for ch in range(NCH):
                    nc.sync.dma_start(out=du_ext[ch], in_=u[ch])
                nc.sync.dma_start(out=ds_ext[:, :, :, :], in_=s_all)

    nc.compile()
    return nc


_NC_CACHE = None
_CALL_CACHE = None


def _make_callable(nc):
    """Build the sharded pjrt callable once (mirrors bass2jax's axon path)."""
    import jax
    import concourse.mybir as mybir
    from concourse import bass2jax
    from jax.sharding import Mesh, PartitionSpec
    from jax.experimental.shard_map import shard_map

    bass2jax.install_neuronx_cc_hook()
    partition_name = (nc.partition_id_tensor.name
                      if nc.partition_id_tensor else None)
    in_names, out_names, out_avals = [], [], []
    for alloc in nc.m.functions[0].allocations:
        if not isinstance(alloc, mybir.MemoryLocationSet):
            continue
        name = alloc.memorylocations[0].name
        if alloc.kind == "ExternalInput":
            if name != partition_name:
                in_names.append(name)
        elif alloc.kind == "ExternalOutput":
            out_names.append(name)
            out_avals.append(jax.core.ShapedArray(
                tuple(alloc.tensor_shape), mybir.dt.np(alloc.dtype)))
    n_params = len(in_names)
    all_in_names = list(in_names) + list(out_names)
    if partition_name:
        all_in_names.append(partition_name)

    def _body(*args):
        operands = list(args)
        if partition_name is not None:
            operands.append(bass2jax.partition_id_tensor())
        outs = bass2jax._bass_exec_p.bind(
            *operands,
            out_avals=tuple(out_avals),
            in_names=tuple(all_in_names),
            out_names=tuple(out_names),
            lowering_input_output_aliases=(),
            sim_require_finite=True,
            sim_require_nnan=True,
            nc=nc,
        )
        return tuple(outs)

    devices = jax.devices()[:NCORES]
    mesh = Mesh(np.asarray(devices), ("core",))
    spec = PartitionSpec("core")
    sharded = jax.jit(shard_map(
        _body, mesh=mesh,
        in_specs=(spec,) * (n_params + len(out_names)),
        out_specs=(spec,) * len(out_names), check_rep=False),
        keep_unused=True)
    zeros = [np.zeros((NCORES * a.shape[0], *a.shape[1:]), a.dtype)
             for a in out_avals]
    return sharded, in_names, out_names, zeros


def kernel(p, a, W_ij):
    global _NC_CACHE, _CALL_CACHE
    import ml_dtypes

    if _NC_CACHE is None:
        _NC_CACHE = _build()
    nc = _NC_CACHE
    if _CALL_CACHE is None:
        _CALL_CACHE = _make_callable(nc)
    sharded, in_names, out_names, zeros = _CALL_CACHE

    p = np.asarray(p, dtype=np.float32).astype(ml_dtypes.bfloat16)
    Wf = np.asarray(W_ij, dtype=np.float32)          # (B, j, k, C)
    wsum = Wf.reshape(128, 128).astype(ml_dtypes.bfloat16)
    wbd = np.zeros((128, B * 128), dtype=np.float32)
    for b in range(B):
        wbd[b * P:(b + 1) * P, b * 128:(b + 1) * 128] = Wf[b].reshape(P, 128)
    wbd = wbd.astype(ml_dtypes.bfloat16)
    ident = np.eye(128, dtype=np.float32)
    per_core = {"p": p.reshape(NCORES * BLOC, B, PP, H, WW),
                "Wbd": np.concatenate([wbd] * NCORES, axis=0),
                "Wsum": np.concatenate([wsum] * NCORES, axis=0),
                "ident": np.concatenate([ident] * NCORES, axis=0)}
    concat_in = [per_core[k] for k in in_names]
    outs = sharded(*concat_in, *zeros)
    res = {name: np.asarray(outs[i]) for i, name in enumerate(out_names)}
    v_full = res["v_out"].reshape(NCORES * BLOC, C, PP, H, WW)
    a_full = res["a_out"].reshape(NCORES * BLOC, C, H, WW)
    return v_full, a_full


# revision 66
# speedup vs baseline: 1.5569x; 1.3256x over previous
"""Capsule dynamic-routing kernel for 8 TRN2 NeuronCores.

Problem: nn_CapsuleRouting — p:(16,32,16,14,14), W_ij:(32,4,4,32), 3 routing
iterations, returns (v:(16,32,16,14,14), a:(16,32,14,14)).

Sharding: data-parallel over batch (2 batch elems per core, 8 cores).

Per-core design:
  - 392 positions (2 batch x 196 hw) in 4 chunks of 98 partitions.
  - SBUF layout: partitions = positions; u votes stored [98, B=32, ik=16, C=32]
    bf16.  Softmax coeffs c[B,C,pos] and squashed votes v[C,ik,pos] broadcast
    along free dims with step-0 APs (no replication needed).
  - Vote einsum on PE with a host-built block-diagonal W (K=128, all B per
    matmul); a plain stacked W contracts (B,j) to give iter0's s0 for free.
  - Reductions over B (weighted sum -> s) and over ik (r update) via PSUM
    accumulation with identity-lhsT matmuls on PE, pipelined behind the DVE
    bf16 2x multiply passes (w2 = c*u or u*v).
  - Small math (softmax exp, squash, reciprocal) on ACT/DVE per 2-chunk
    group so groups pipeline against each other's big passes.
"""

import os
import numpy as np
from contextlib import ExitStack
EXP = os.environ.get("KEXP", "")

P = 4
PP = 16
B = 32
C = 32
H = 14
WW = 14
POS = H * WW          # 196
HALF = POS // 2       # 98
BLOC = 2              # batch elems per core
NCH = 4               # chunks per core: (batch, half)
GRP = 1               # chunks per small-op group
ITERS = 3
EPS = 1e-5
NCORES = 8


def _build(debug=False, reps=1, serialize=False):
    import concourse.bass as bass
    import concourse.bacc as bacc
    import concourse.tile as tile
    from concourse import mybir

    f32 = mybir.dt.float32
    bf16 = mybir.dt.bfloat16
    AX = mybir.AxisListType
    AF = mybir.ActivationFunctionType

    nc = bacc.Bacc()
    p_ext = nc.declare_dram_parameter("p", [BLOC, B, PP, H, WW], bf16, isOutput=False)
    wbd_ext = nc.declare_dram_parameter("Wbd", [128, B * 128], bf16, isOutput=False)
    wsum_ext = nc.declare_dram_parameter("Wsum", [128, 128], bf16, isOutput=False)
    id_ext = nc.declare_dram_parameter("ident", [128, 128], f32, isOutput=False)
    v_ext = nc.declare_dram_parameter("v_out", [BLOC, C, PP, POS], f32, isOutput=True)
    a_ext = nc.declare_dram_parameter("a_out", [BLOC, C, POS], f32, isOutput=True)
    if debug:
        du_ext = nc.declare_dram_parameter(
            "dbg_u", [NCH, 98, B, PP, C], bf16, isOutput=True)
        ds_ext = nc.declare_dram_parameter(
            "dbg_s", [98, NCH, PP, C], f32, isOutput=True)

    def bcast(sl, axis, count):
        """Insert a step-0 (broadcast) dim into AP `sl` at free-dim position
        `axis` (0 = first free dim)."""
        ap = list(sl.ap)
        ap.insert(1 + axis, [0, count])
        return bass.AP(tensor=sl.tensor, offset=sl.offset, ap=ap)

    with tile.TileContext(nc) as tc, ExitStack() as ctx:
        singles = ctx.enter_context(tc.tile_pool(name="singles", bufs=1))
        upool = ctx.enter_context(tc.tile_pool(name="upool", bufs=4))
        w2pool = ctx.enter_context(tc.tile_pool(name="w2pool", bufs=3))
        small = ctx.enter_context(tc.tile_pool(name="small", bufs=1))
        outp = ctx.enter_context(tc.tile_pool(name="outp", bufs=2))
        mmps = ctx.enter_context(tc.tile_pool(name="mmps", bufs=2, space="PSUM"))
        sps = ctx.enter_context(tc.tile_pool(name="sps", bufs=3, space="PSUM"))
        tps = ctx.enter_context(tc.tile_pool(name="tps", bufs=1, space="PSUM"))

        # --- constants ---
        id_f = singles.tile([128, 128], f32)
        nc.sync.dma_start(out=id_f, in_=id_ext[:, :])
        id_b = singles.tile([128, 128], bf16)
        nc.scalar.copy(id_b, id_f)
        idb98 = id_b[:98, :98]
        idf98 = id_f[:98, :98]
        eps_t = singles.tile([98, 1], f32)
        nc.vector.memset(eps_t, EPS)
        wsum = singles.tile([128, 128], bf16)
        nc.sync.dma_start(out=wsum, in_=wsum_ext[:, :])

        # --- persistent state (chunk index is a free dim; ops slice groups) ---
        u = [upool.tile([98, B, PP, C], bf16, tag="u", name=f"u{i}")
             for i in range(NCH)]
        r_all = small.tile([98, NCH, B, C], bf16)      # routing logits
        c_all = small.tile([98, NCH, B, C], bf16)      # softmax coeffs
        s_all = small.tile([98, NCH, PP, C], f32)      # s, then v (in-place)
        vbf_all = small.tile([98, NCH, PP, C], bf16)   # v in bf16
        n2_all = small.tile([98, NCH, C], f32)
        nrm_all = small.tile([98, NCH, C], f32)
        sc_all = small.tile([98, NCH, C], f32)
        d_all = small.tile([98, NCH, B], f32)

        def squash(it, c0, cn, sp_tiles=None):
            """s -> v (into s_all) for chunks [c0, c0+cn); sets n2/nrm/sc.
            With sp_tiles, reads s from the per-chunk accum PSUM directly
            (skips the ACT psum->SBUF copy on the critical chain)."""
            ssl = s_all[:, c0:c0 + cn, :, :]
            # scratch for squares: alias the dead B<16 half of c_all (current
            # iter's c is already consumed; next softmax rewrites it fully)
            sq = c_all[:, c0:c0 + cn, :PP, :]
            scale = (1.0 / B) if it == 0 else 1.0
            n2 = n2_all[:, c0:c0 + cn, :]
            for k, sp in enumerate(sp_tiles):
                nc.scalar.activation(sq[:, k, :, :], sp, AF.Square,
                                     scale=scale)
                nc.vector.tensor_reduce(
                    n2_all[:, c0 + k:c0 + k + 1, :],
                    sq[:, k:k + 1, :, :].rearrange("q n i c -> q n c i"),
                    axis=AX.X, op=mybir.AluOpType.add,
                )
            nrm = nrm_all[:, c0:c0 + cn, :]
            sc = sc_all[:, c0:c0 + cn, :]
            nc.scalar.activation(nrm, n2, AF.Sqrt, bias=eps_t)
            nc.gpsimd.tensor_scalar_add(sc, n2, 1.0)
            nc.gpsimd.tensor_mul(sc, sc, nrm)          # (1+n2)*nrm
            nc.vector.reciprocal(sc, sc)
            nc.gpsimd.tensor_mul(sc, sc, n2)           # n2/((1+n2)nrm)
            last = it == ITERS - 1
            if it == 0:
                # v = (s0_psum/32) * sc: fold 1/32 into a scaled sc copy
                sc32 = d_all[:, c0:c0 + cn, :]
                nc.scalar.activation(sc32, sc, AF.Copy, scale=1.0 / B)
                for k, sp in enumerate(sp_tiles):
                    nc.vector.tensor_mul(
                        vbf_all[:, c0 + k, :, :], sp,
                        bcast(d_all[:, c0 + k, :], 0, PP))
            else:
                for k, sp in enumerate(sp_tiles):
                    nc.vector.tensor_mul(
                        (s_all if last else vbf_all)[:, c0 + k, :, :], sp,
                        bcast(sc_all[:, c0 + k, :], 0, PP))

        def softmax(c0, cn):
            rsl = r_all[:, c0:c0 + cn, :, :]
            csl = c_all[:, c0:c0 + cn, :, :]
            d = d_all[:, c0:c0 + cn, :]
            nc.scalar.activation(csl, rsl, AF.Exp)
            nc.vector.tensor_reduce(d, csl, axis=AX.X, op=mybir.AluOpType.add)
            nc.vector.reciprocal(d, d)
            nc.gpsimd.tensor_mul(csl, csl, bcast(d, 2, C))

        def emit_outputs(c0, cn):
            asl = nrm_all[:, c0:c0 + cn, :]
            # a = sqrt((n2/(1+n2))^2 + eps);  n2/(1+n2) = sc*nrm
            nc.vector.tensor_mul(asl, sc_all[:, c0:c0 + cn, :], asl)
            nc.vector.tensor_mul(asl, asl, asl)
            nc.scalar.activation(asl, asl, AF.Sqrt, bias=eps_t)
            for ch in range(c0, c0 + cn):
                bi, hf = divmod(ch, 2)
                vdst = v_ext[bi].rearrange("c (i k) s -> k c i s", i=P, k=P)
                for q in range(P):
                    tp = tps.tile([128, 98], f32, tag="tp")
                    nc.tensor.transpose(
                        tp,
                        s_all[:, ch, q * P:(q + 1) * P, :]
                        .rearrange("q a b -> q (a b)"),
                        idf98,
                    )
                    vt = outp.tile([128, 98], f32, tag="vt")
                    nc.scalar.copy(vt, tp)
                    nc.sync.dma_start(
                        out=vdst[:, :, q, hf * HALF:(hf + 1) * HALF],
                        in_=vt,
                    )
                tp = tps.tile([128, 98], f32, tag="tp")
                nc.tensor.transpose(tp[:C, :], nrm_all[:, ch, :], idf98)
                at = outp.tile([128, 98], f32, tag="vt")
                nc.scalar.copy(at[:C, :], tp[:C, :])
                nc.sync.dma_start(
                    out=a_ext[bi, :, hf * HALF:(hf + 1) * HALF],
                    in_=at[:C, :],
                )

        for _rep in range(reps):
            # --- phase 1: vote einsum ---
            if True:
                wbd = w2pool.tile([128, B * 128], bf16, tag="w2",
                                  name=f"wbd{_rep}")
                nc.sync.dma_start(out=wbd, in_=wbd_ext[:, :])
                s0_tiles = []
                if serialize and _rep > 0:
                    # hard cross-rep serializer: wbd corner <- 0 * prev output
                    # (unused id_f corner as bounce; outside the idf98 slice)
                    nc.sync.dma_start(out=id_f[0:1, 120:128],
                                      in_=a_ext[0, 0, 0:8])
                    nc.vector.tensor_scalar_mul(
                        wbd[0:1, 512:520],
                        id_f[0:1, 120:124].bitcast(bf16), 0.0)
                for ch in range(NCH):
                    bi, hf = divmod(ch, 2)
                    src = p_ext[bi].rearrange(
                        "b (i j) h w -> b j i (h w)", i=P, j=P)
                    p_b = w2pool.tile([128, P, HALF], bf16, tag="w2",
                                      name=f"pb{_rep}_{ch}")
                    for i in range(P):
                        nc.sync.dma_start(
                            out=p_b[:, i, :],
                            in_=src[:, :, i, hf * HALF:(hf + 1) * HALF],
                        )
                    s0 = sps.tile([98, P, 128], f32, tag="sp",
                                  name=f"s0_{ch}")
                    for i in range(P):
                        nc.tensor.matmul(s0[:, i, :], p_b[:, i, :], wsum,
                                         start=True, stop=True)
                    s0_tiles.append(s0.rearrange("q i n -> q (i n)"))
                    for i in range(P):
                        for hp in range(4):
                            ps = mmps.tile([98, 8, P, C], f32, tag="mm")
                            for t in range(2):
                                pc = hp * 2 + t
                                nc.tensor.matmul(
                                    ps[:, t * 4:(t + 1) * 4, :, :],
                                    p_b[:, i, :],
                                    wbd[:, pc * 512:(pc + 1) * 512],
                                    start=True, stop=True,
                                )
                            dst = u[ch][:, hp * 8:(hp + 1) * 8,
                                        i * P:(i + 1) * P, :]
                            if hp % 2 == 0:
                                nc.vector.tensor_copy(dst, ps)
                            else:
                                nc.scalar.copy(dst, ps)

            # --- routing iterations (iter0 s came from the einsum) ---
            for it in range(ITERS):
                grp = 1 if it == ITERS - 1 else GRP
                for g2 in range(NCH // grp):
                    chs = range(g2 * grp, (g2 + 1) * grp)
                    if it == 0:
                        sp_tiles = [s0_tiles[ch] for ch in chs]
                    else:
                        sp_tiles = []
                        SUBS = [(0, 8), (8, 8), (16, 16)]
                        for ch in chs:
                            sp = sps.tile([98, PP, C], f32, tag="sp",
                                          name=f"sp{it}_{ch}")
                            sp_tiles.append(sp)
                            for si, (b0, sz) in enumerate(SUBS):
                                w2 = w2pool.tile([98, sz, PP, C], bf16,
                                                 tag="w2", name=f"w2s{ch}{si}")
                                csl = c_all[:, ch, b0:b0 + sz, :]
                                nc.vector.tensor_mul(
                                    w2, u[ch][:, b0:b0 + sz, :, :],
                                    bcast(csl, 1, PP),
                                )
                                for bb in range(sz):
                                    nc.tensor.matmul(
                                        sp[:, :, :], idb98, w2[:, bb, :, :],
                                        start=(b0 == 0 and bb == 0),
                                        stop=(b0 + bb == B - 1),
                                    )
                    squash(it, g2 * grp, grp, sp_tiles)
                    if it == ITERS - 1:
                        emit_outputs(g2 * grp, grp)
                        continue
                    for ch in chs:
                        dp = mmps.tile([98, B, C], f32, tag="mm",
                                       name=f"dp{it}_{ch}")
                        for sb in range(2):
                            w2 = w2pool.tile([98, 16, PP, C], bf16,
                                             tag="w2", name=f"w2d{ch}{sb}")
                            vsl = vbf_all[:, ch, :, :]
                            if "nomul" in EXP:
                                nc.vector.tensor_mul(
                                    w2[:, 0, :, :],
                                    u[ch][:, sb * 16, :, :], vsl)
                            else:
                                nc.vector.tensor_mul(
                                    w2, u[ch][:, sb * 16:(sb + 1) * 16, :, :],
                                    bcast(vsl, 0, 16),
                                )
                            for ik in range(PP):
                                if "noacc" in EXP and ik > 0:
                                    continue
                                nc.tensor.matmul(
                                    dp[:, sb * 16:(sb + 1) * 16, :],
                                    idb98, w2[:, :, ik, :],
                                    start=(ik == 0),
                                    stop=True if "noacc" in EXP else
                                    (ik == PP - 1),
                                )
                        if it == 0:
                            nc.scalar.copy(r_all[:, ch, :, :], dp)
                            nc.scalar.activation(
                                c_all[:, ch, :, :], dp, AF.Exp)
                        else:
                            nc.vector.tensor_add(
                                r_all[:, ch, :, :], r_all[:, ch, :, :], dp)
                    if it == 0:
                        c0s = g2 * grp
                        csl = c_all[:, c0s:c0s + grp, :, :]
                        d = d_all[:, c0s:c0s + grp, :]
                        nc.vector.tensor_reduce(
                            d, csl, axis=AX.X, op=mybir.AluOpType.add)
                        nc.vector.reciprocal(d, d)
                        nc.gpsimd.tensor_mul(csl, csl, bcast(d, 2, C))
                    else:
                        softmax(g2 * grp, grp)


            if debug:
                for ch in range(NCH):
                    nc.sync.dma_start(out=du_ext[ch], in_=u[ch])
                nc.sync.dma_start(out=ds_ext[:, :, :, :], in_=s_all)

    nc.compile()
    return nc


_NC_CACHE = None
_CALL_CACHE = None


def _make_callable(nc):
    """Build the sharded pjrt callable once (mirrors bass2jax's axon path)."""
    import jax
    import concourse.mybir as mybir
    from concourse import bass2jax
    from jax.sharding import Mesh, PartitionSpec
    from jax.experimental.shard_map import shard_map

    bass2jax.install_neuronx_cc_hook()
    partition_name = (nc.partition_id_tensor.name
                      if nc.partition_id_tensor else None)
    in_names, out_names, out_avals = [], [], []
    for alloc in nc.m.functions[0].allocations:
        if not isinstance(alloc, mybir.MemoryLocationSet):
            continue
        name = alloc.memorylocations[0].name
        if alloc.kind == "ExternalInput":
            if name != partition_name:
                in_names.append(name)
        elif alloc.kind == "ExternalOutput":
            out_names.append(name)
            out_avals.append(jax.core.ShapedArray(
                tuple(alloc.tensor_shape), mybir.dt.np(alloc.dtype)))
    n_params = len(in_names)
    all_in_names = list(in_names) + list(out_names)
    if partition_name:
        all_in_names.append(partition_name)

    def _body(*args):
        operands = list(args)
        if partition_name is not None:
            operands.append(bass2jax.partition_id_tensor())
        outs = bass2jax._bass_exec_p.bind(
            *operands,
            out_avals=tuple(out_avals),
            in_names=tuple(all_in_names),
            out_names=tuple(out_names),
            lowering_input_output_aliases=(),
            sim_require_finite=True,
            sim_require_nnan=True,
            nc=nc,
        )
        return tuple(outs)

    devices = jax.devices()[:NCORES]
    mesh = Mesh(np.asarray(devices), ("core",))
    spec = PartitionSpec("core")
    sharded = jax.jit(shard_map(
        _body, mesh=mesh,
        in_specs=(spec,) * (n_params + len(out_names)),
        out_specs=(spec,) * len(out_names), check_rep=False),
        keep_unused=True)
    zeros = [np.zeros((NCORES * a.shape[0], *a.shape[1:]), a.dtype)
             for a in out_avals]
    return sharded, in_names, out_names, zeros


def kernel(p, a, W_ij):
    global _NC_CACHE, _CALL_CACHE
    import ml_dtypes

    if _NC_CACHE is None:
        _NC_CACHE = _build()
    nc = _NC_CACHE
    if _CALL_CACHE is None:
        _CALL_CACHE = _make_callable(nc)
    sharded, in_names, out_names, zeros = _CALL_CACHE

    p = np.asarray(p, dtype=np.float32).astype(ml_dtypes.bfloat16)
    Wf = np.asarray(W_ij, dtype=np.float32)          # (B, j, k, C)
    wsum = Wf.reshape(128, 128).astype(ml_dtypes.bfloat16)
    wbd = np.zeros((128, B * 128), dtype=np.float32)
    for b in range(B):
        wbd[b * P:(b + 1) * P, b * 128:(b + 1) * 128] = Wf[b].reshape(P, 128)
    wbd = wbd.astype(ml_dtypes.bfloat16)
    ident = np.eye(128, dtype=np.float32)
    per_core = {"p": p.reshape(NCORES * BLOC, B, PP, H, WW),
                "Wbd": np.concatenate([wbd] * NCORES, axis=0),
                "Wsum": np.concatenate([wsum] * NCORES, axis=0),
                "ident": np.concatenate([ident] * NCORES, axis=0)}
    concat_in = [per_core[k] for k in in_names]
    outs = sharded(*concat_in, *zeros)
    res = {name: np.asarray(outs[i]) for i, name in enumerate(out_names)}
    v_full = res["v_out"].reshape(NCORES * BLOC, C, PP, H, WW)
    a_full = res["a_out"].reshape(NCORES * BLOC, C, H, WW)
    return v_full, a_full
